# revision 25
# baseline (speedup 1.0000x reference)
"""Causal MHA (B=4, S=4096, D=64, scale=1/sqrt(S)) on 8 trn2 NeuronCores.

Strategy (identical SPMD program on all 8 cores; per-core data differs):
  - scale = 1/sqrt(4096) = 1/64 (reference scales by sqrt of SEQ length).
  - scoresT layout [k, q]: softmax reduction (over k) becomes a matmul
    contraction; AV needs no transposes anywhere (host pre/post transposes).
  - Each core: one batch b = c%4, eight query slots of 256 rows. Slot m
    (1..8) covers query block j_m = 2m-1-(c//4) (q0 = 256*j_m) and iterates
    4m k-tiles of 128 keys. Sorted-descending capacity pairing makes every
    core's slot multiset identical (uniform program) with ~6% padding waste,
    masked to zero where invalid.
  - QK: scoresT[k,q] += KT_tile.T @ QT, contraction over d=64. k-tiles packed
    into partition halves [0:64)/[64:128) so adjacent k-tiles run concurrently
    in disjoint PE row groups. bf16 inputs (score err ~5e-4 post-scale).
  - exp via ScalarE ACTIVATE(Exp, scale=1/64) straight out of PSUM, one op
    per <=6 k-tiles [128,1536], bf16 out to SBUF.
  - causal/padding masks: host-built 0/1 bf16, DVE multiply on the last 4
    k-tiles of each slot only.
  - AV: outT[d,q] += V_tile(+ones col).T @ expT, fp32 PSUM accumulation over
    all k-tiles of the slot. The ones column makes row 64 the softmax
    denominator - normalization happens on host (free).
  - Output per core: OT [65, 2048] fp32; host divides and scatters.

Perf structure (ScalarE exp is the wall: ~37.6us busy of ~55us total):
  - PE warm-up burst at t=0 (dummy matmuls on a memset tile) so the HAM
    clock gate reaches 2.4 GHz by ~11us instead of ~22us in.
  - Input DMAs split and ordered by first use; no global wait-ladder (the
    v1 ladder blocked the PE queue on DMAs not needed until much later).
  - ACT chunks of up to 6 k-tiles (27 ACTIVATEs/core instead of 36).
  - AV matmuls deferred by one chunk ACROSS slot boundaries, masked tail
    AVs by two, so neither the exp of the chunk just issued nor the DVE
    mask multiply is ever on the ACT->AV critical path (ACT runs b2b).
  - Each slot's 4-tile causal mask is ONE DVE multiply: the perms put the
    masked tiles on contiguous PSUM columns; host supplies both column
    orders of the mask blocks (cols 0:1024 for 6-chunks, 1024:2048 for the
    perm4 layout of slot 1's 4-chunk).
Measured 55.0-57.3us (baseline 63.9us). Known-good; do NOT reintroduce a
2-tile lead chunk with perm [0,1]: two concurrent row-group matmuls
draining into one PSUM bank hang the device (needed a core reset).
"""

import sys

sys.path.insert(0, "/opt/trn_rl_repo")

import numpy as np
import ml_dtypes

B, S, D = 4, 4096, 64
NCORES = 8
NSLOTS = 8          # query slots per core, 256 queries each
QS = 256            # queries per slot
KT_TILE = 128       # keys per k-tile
NKT = S // KT_TILE  # 32 k-tiles per batch
BF16 = ml_dtypes.bfloat16

_COMPILED = None

WARM_MMS = 5        # PE warm-up matmuls (N=512 each) issued before real work
_CACHE_BUST = 11    # bump to force a NEFF recompile


def chunk_sizes(T, first_slot=False):
    """Split T (=4m) k-tiles into chunks that END in a 6 (except T=4) so the
    4 masked tail tiles sit at contiguous PSUM columns under perm6.

    (A 2-tile lead chunk with perm [0,1] was tried to start the exp train
    earlier; the two concurrent row-group matmuls draining into the same
    PSUM bank hung the hardware. Do not reintroduce without bank-disjoint
    columns.)"""
    if T == 4:
        return [4]
    if T == 8:
        return [4, 4]
    rem = T % 6
    pre = [] if rem == 0 else ([4] if rem == 4 else [4, 4])
    return pre + [6] * ((T - sum(pre)) // 6)


def _build_program():
    import concourse.bacc as bacc
    import concourse.tile as tile
    import concourse.mybir as mybir

    F32 = mybir.dt.float32
    MBF16 = mybir.dt.bfloat16
    EXPF = mybir.ActivationFunctionType.Exp

    nc = bacc.Bacc("TRN2", target_bir_lowering=False, debug=False, num_devices=NCORES)

    ktp = nc.dram_tensor("ktp", [128, 16 * 128], MBF16, kind="ExternalInput").ap()
    qtd = nc.dram_tensor("qtd", [128, NSLOTS * QS], MBF16, kind="ExternalInput").ap()
    va = nc.dram_tensor("va", [128, NKT * 65], MBF16, kind="ExternalInput").ap()
    mask = nc.dram_tensor("mask", [128, 8 * QS], MBF16, kind="ExternalInput").ap()
    ot = nc.dram_tensor("ot", [65, NSLOTS * QS], F32, kind="ExternalOutput").ap()

    # column permutation inside a PSUM chunk: concurrent row-paired matmuls
    # (j,j+1) land in different PSUM banks AND the 4 masked tail tiles land
    # on contiguous columns (cols 1-4 under perm6; cols 0-3 under perm4)
    # so each slot's causal mask is a single DVE multiply.
    # NOTE: the ACT covers exactly cols [0, csize*QS), so a perm must be a
    # bijection onto {0..csize-1} — a sparse perm leaves un-exp'd columns.
    perms = {
        4: [0, 2, 1, 3],
        6: [0, 5, 1, 2, 3, 4],
    }

    with tile.TileContext(nc) as tc:
        with (
            tc.tile_pool(name="ins", bufs=1) as ins,
            tc.tile_pool(name="work", bufs=3) as work,
            tc.tile_pool(name="outs", bufs=2) as outs,
            tc.tile_pool(name="ps", bufs=1, space="PSUM") as ps,
            tc.tile_pool(name="pso", bufs=1, space="PSUM") as pso,
        ):
            ktp_sb = ins.tile([128, 16 * 128], MBF16)
            qtd_sb = ins.tile([128, NSLOTS * QS], MBF16)
            va_sb = ins.tile([128, NKT * 65], MBF16)
            mask_sb = ins.tile([128, 8 * QS], MBF16)
            warm_sb = ins.tile([128, 512], MBF16)

            # PE warm-up: memset a tile, then a burst of back-to-back matmuls
            # with no other dependencies. They run while input DMAs are in
            # flight and trip the HAM busy-window so the PE clock is at
            # 2.4 GHz by the time real matmuls start.
            nc.gpsimd.memset(warm_sb, 0.25)
            p_warm = ps.tile([128, 6 * QS], F32, tag="scores", bufs=2)
            for _ in range(WARM_MMS):
                nc.tensor.matmul(
                    p_warm[:, :512],
                    warm_sb[:, :128],
                    warm_sb[:, :512],
                    start=True,
                    stop=True,
                )

            # Input DMAs: split + ordered by first consumption. Each lands on
            # its own HW queue; the Sync engine issues descriptors serially in
            # this order (~0.7us each).
            nc.sync.dma_start(out=qtd_sb[:, 7 * QS :], in_=qtd[:, 7 * QS :])  # slot 8
            nc.sync.dma_start(out=ktp_sb[:, :512], in_=ktp[:, :512])        # k-tiles 0-7
            nc.sync.dma_start(out=va_sb[:, : 8 * 65], in_=va[:, : 8 * 65])  # v-tiles 0-7
            nc.sync.dma_start(out=ktp_sb[:, 512:], in_=ktp[:, 512:])
            nc.sync.dma_start(out=va_sb[:, 8 * 65 :], in_=va[:, 8 * 65 :])
            nc.sync.dma_start(out=qtd_sb[:, 4 * QS : 7 * QS], in_=qtd[:, 4 * QS : 7 * QS])
            nc.sync.dma_start(out=mask_sb, in_=mask)
            nc.sync.dma_start(out=qtd_sb[:, : 4 * QS], in_=qtd[:, : 4 * QS])

            # AV jobs deferred by chunk count: unmasked tiles run one chunk
            # after their exp, masked tail tiles two chunks after (so the DVE
            # mask multiply is never on the ACT->AV critical path).
            av_jobs = []  # (ready_chunk, t, rhs_ap, p_out, T, qlo)

            def emit_ready(now):
                rest = []
                for jb in av_jobs:
                    ready, t, rhs_ap, jp_out, jT, jqlo = jb
                    if ready <= now:
                        nc.tensor.matmul(
                            jp_out,
                            va_sb[:, 65 * t : 65 * t + 65],
                            rhs_ap,
                            start=(t == 0),
                            stop=(t == jT - 1),
                        )
                        if t == jT - 1:
                            # slot finished: drain PSUM accumulator + write out
                            o_sb = outs.tile([65, QS], F32, tag="drain")
                            nc.vector.tensor_copy(o_sb, jp_out)
                            nc.sync.dma_start(out=ot[:, jqlo : jqlo + QS], in_=o_sb)
                    else:
                        rest.append(jb)
                av_jobs[:] = rest

            g = 0  # global chunk counter
            # slots descending: deep pipelines first, 1-chunk slot last
            for m in range(NSLOTS, 0, -1):
                qlo = (m - 1) * QS
                T = 4 * m  # k-tiles this slot
                p_out = pso.tile([65, QS], F32, tag="avout", bufs=2)
                chunks = chunk_sizes(T, first_slot=(m == NSLOTS))
                base = 0
                for ci, csize in enumerate(chunks):
                    perm = perms[csize]
                    last = ci == len(chunks) - 1
                    p_sc = ps.tile([128, 6 * QS], F32, tag="scores", bufs=2)
                    for j in range(csize):
                        t = base + j
                        h = t % 2
                        u = t // 2
                        nc.tensor.matmul(
                            p_sc[:, perm[j] * QS : perm[j] * QS + QS],
                            ktp_sb[64 * h : 64 * h + 64, 128 * u : 128 * u + 128],
                            qtd_sb[64 * h : 64 * h + 64, qlo : qlo + QS],
                            start=True,
                            stop=True,
                        )
                    e_sb = work.tile([128, 6 * QS], MBF16, tag="expT")
                    nc.scalar.activation(
                        e_sb[:, : csize * QS], p_sc[:, : csize * QS], EXPF,
                        scale=1.0 / 64.0,
                    )
                    rhs = [
                        e_sb[:, perm[j] * QS : perm[j] * QS + QS] for j in range(csize)
                    ]
                    if last:
                        # causal/padding mask over the final 4 k-tiles: one DVE
                        # multiply over their contiguous column range.
                        mk_sb = work.tile([128, 4 * QS], MBF16, tag="maskedT", bufs=2)
                        if csize == 6:
                            nc.vector.tensor_mul(
                                mk_sb, e_sb[:, QS : 5 * QS], mask_sb[:, : 4 * QS]
                            )
                            for mj in range(4):
                                rhs[2 + mj] = mk_sb[:, mj * QS : mj * QS + QS]
                        else:  # csize == 4 (slot 1): perm4-ordered mask copy
                            nc.vector.tensor_mul(
                                mk_sb, e_sb[:, : 4 * QS], mask_sb[:, 4 * QS :]
                            )
                            for mj in range(4):
                                rhs[mj] = mk_sb[:, perm[mj] * QS : perm[mj] * QS + QS]
                    for j in range(csize):
                        t = base + j
                        masked = last and j >= csize - 4
                        av_jobs.append((g + (2 if masked else 1), t, rhs[j], p_out, T, qlo))
                    emit_ready(g)
                    base += csize
                    g += 1
            emit_ready(1 << 30)

    nc.compile()
    return nc


def _get_compiled():
    global _COMPILED
    if _COMPILED is None:
        _COMPILED = _build_program()
    return _COMPILED


def _make_masks(half):
    ki = np.arange(KT_TILE)[:, None]
    qj = np.arange(QS)[None, :]
    d_a = (qj >= ki).astype(np.float32)
    d_b = (qj >= ki + 128).astype(np.float32)
    ones = np.ones((KT_TILE, QS), np.float32)
    zeros = np.zeros((KT_TILE, QS), np.float32)
    m = [ones, ones, d_a, d_b] if half == 0 else [d_a, d_b, zeros, zeros]
    # cols 0:1024 in tile order (6-final chunks, masked tiles at cols 1-4);
    # cols 1024:2048 permuted by perm4=[0,2,1,3] (slot 1's 4-tile chunk).
    return np.concatenate([m[0], m[1], m[2], m[3], m[0], m[2], m[1], m[3]], axis=1)


def make_in_maps(Q, K, V):
    """Pack full fp32 Q,K,V [B,S,D] into 8 per-core input dicts."""
    in_maps = []
    for c in range(NCORES):
        b = c % 4
        half = c // 4
        # KT packed: k-tile t -> partition half t%2, cols 128*(t//2)
        kt = np.ascontiguousarray(K[b].T)  # [64, 4096]
        ktp = np.empty((128, 16 * 128), np.float32)
        for t in range(NKT):
            h, u = t % 2, t // 2
            ktp[64 * h : 64 * h + 64, 128 * u : 128 * u + 128] = kt[
                :, 128 * t : 128 * t + 128
            ]
        # Q slots (duplicated into both partition halves)
        qrows = np.concatenate(
            [Q[b, 256 * (2 * m - 1 - half) : 256 * (2 * m - 1 - half) + 256] for m in range(1, 9)],
            axis=0,
        )  # [2048, 64]
        qt = np.ascontiguousarray(qrows.T)  # [64, 2048]
        qtd = np.concatenate([qt, qt], axis=0)  # [128, 2048]
        # V augmented with ones column, tiles side by side
        va = np.empty((128, NKT * 65), np.float32)
        for t in range(NKT):
            va[:, 65 * t : 65 * t + 64] = V[b, 128 * t : 128 * t + 128, :]
            va[:, 65 * t + 64] = 1.0
        in_maps.append(
            {
                "ktp": ktp.astype(BF16),
                "qtd": qtd.astype(BF16),
                "va": va.astype(BF16),
                "mask": _make_masks(half).astype(BF16),
            }
        )
    return in_maps


def unpack_outputs(results):
    """Combine 8 per-core OT [65, 2048] fp32 into full output [B,S,D]."""
    out = np.empty((B, S, D), np.float32)
    for c in range(NCORES):
        b = c % 4
        half = c // 4
        otc = results[c]["ot"]  # [65, 2048]
        for m in range(1, 9):
            j = 2 * m - 1 - half
            sl = otc[:, 256 * (m - 1) : 256 * m]  # [65, 256]
            out[b, 256 * j : 256 * j + 256, :] = (sl[:64] / sl[64:65]).T
    return out


def run_on_hw(in_maps, trace=False, trace_cores=None):
    from concourse.bass_utils import run_bass_kernel_spmd

    nc = _get_compiled()
    return run_bass_kernel_spmd(
        nc, in_maps, core_ids=list(range(NCORES)), trace=trace, trace_cores=trace_cores
    )


def kernel(Q, K, V):
    Q = np.asarray(Q, np.float32)
    K = np.asarray(K, np.float32)
    V = np.asarray(V, np.float32)
    res = run_on_hw(make_in_maps(Q, K, V), trace=False)
    return unpack_outputs(res.results)
